# revision 41
# baseline (speedup 1.0000x reference)
"""Neural CDE (RK4, piecewise-constant path derivative) Trainium2 kernel.

Pure batch parallelism, B=128 -> 16 per core across 8 cores. State kept
feature-major in split form (top[64]+bottom[64]) interleaved into the tail
tile: ge[p, j*9+c] holds tanh outputs for chunks c<8 and the fp32 state for
c==8, so the q-contraction is one fully-linear multiply + one linear
segmented reduce. The (hb, hr) fp16 pairs live in single [128,32] tiles and
the two Wb terms of every layer ride ONE N=32 matmul whose output access
pattern repeats the same PSUM addresses (both halves accumulate in a single
instruction), halving the hr-stall restarts and 8 matmuls per stage.

Matmuls run in double-fp16, 3 terms: Wb.hb + Wr.hb + Wb.hr (the Wr.hr term
is ~2^-22 relative, dropped). fp16 halves the residual magnitude vs bf16
(2^-11 vs 2^-8), and the 3-term scheme keeps the recurrence at fp32 grade
(CPU-sim rel err 3e-4 over the 512-step horizon vs 2e-2 tolerance).

Biases are pre-seeded into PSUM by K=128 FWL-rate matmuls, so each relu
boundary is hb = relu(psum) (fp16 out) and hr = max(psum,0) - hb on the
vector engine. RK4 bookkeeping is emitted after the next stage's relu so
its vector-queue position never delays the state-tile semaphore the PE
waits on. The tail (tanh -> multiply -> reduce) is fp32 throughout (any
16-bit rounding there amplifies ~350x over the 512-step horizon).
"""

import os
import sys
from contextlib import ExitStack

import numpy as np
import ml_dtypes

sys.path.insert(0, "/opt/trn_rl_repo")

import concourse.bass as bass
import concourse.tile as tile
from concourse import bacc
from concourse import mybir
from concourse.bass_utils import run_bass_kernel_spmd

B, L, X, Z, H = 128, 512, 16, 64, 128
NCORES = 8
BPC = B // NCORES  # 16
DT = 0.1
F32 = mybir.dt.float32
F16 = mybir.dt.float16
AF = mybir.ActivationFunctionType
OP = mybir.AluOpType

# chunk permutation: psum column group c holds original W3 column z*16+x
# with x = 2c + (p>=64), z = p%64 (p = output partition)
_p = np.arange(128)
_c = np.arange(8)
ORIG_COL = (_p[None, :] % 64) * 16 + 2 * _c[:, None] + (_p[None, :] // 64)  # [8,128]


def _v3(t):
    return t[:].rearrange("p (j c) -> p j c", c=9)


def slot_ap(t):
    """strided state-slot view: positions j*9+8 within a [128,144] tile."""
    return _v3(t)[:, :, 8]


def grid_ap(t):
    """3D view of the 128 tanh positions j*9+c (j outer, c inner)."""
    return _v3(t)[:, :, 0:8]


def chunk_ap(t, c):
    """strided chunk view: positions j*9+c, 16 elements."""
    return _v3(t)[:, :, c]


def build_nc(l_steps=L):
    nc = bacc.Bacc("TRN2")

    dp = nc.declare_dram_parameter
    # wf rows: per half h: 144 fp32 values val(h, j*9+c); c<8: dt*v[2c+h, j],
    # c==8: -0.001*dt*sum_x v
    vsmall = dp("vsmall", [l_steps, 288], F32, isOutput=False).ap()
    # [w1b|w1r|w2b|w2r|w3b(1024)|w3r(1024)] all fp16, single DMA
    wmm_d = dp("wmm", [128, 2560], F16, isOutput=False).ap()
    # [B3MAT | SEL128] fp16: K=128 seed weights (rows 2c+t = b3{b,r}[c]) and
    # selector moving (row 2c+t active for chunk c) -> FWL-rate weight loads
    b3sel_d = dp("b3sel", [128, 256], F16, isOutput=False).ap()
    # [B1MAT/32 | B2MAT/32 | ones512] fp16: K=128 bias-seed weights (rows
    # 0/1 = bb/32, br/32; exact exponent shift). The N=512 seed matmul
    # accumulates the bias 32x via a repeated-address output AP, keeping the
    # PE streaming through the relu bubble so the next matmul's weight load
    # pulls ahead instead of cold-serializing.
    b12_d = dp("b12", [128, 768], F16, isOutput=False).ap()
    wi1x_d = dp("wi1x", [16, 144], F32, isOutput=False).ap()  # [wi1 | x0t]
    wi2_d = dp("wi2", [128, 128], F32, isOutput=False).ap()
    wi3_d = dp("wi3", [128, 64], F32, isOutput=False).ap()
    bi1_d = dp("bi1", [128, 1], F32, isOutput=False).ap()
    bi2_d = dp("bi2", [128, 1], F32, isOutput=False).ap()
    bi3_d = dp("bi3", [64, 1], F32, isOutput=False).ap()
    # split-form state per step (fp16 copy; host folds top+bottom halves)
    zall = dp("zall", [l_steps, 128, BPC], F16, isOutput=True).ap()

    with tile.TileContext(nc) as tc, ExitStack() as ctx:
        singles = ctx.enter_context(tc.tile_pool(name="singles", bufs=1))
        wfp = ctx.enter_context(tc.tile_pool(name="wfp", bufs=4))
        gep = ctx.enter_context(tc.tile_pool(name="gep", bufs=4))
        mp = ctx.enter_context(tc.tile_pool(name="mp", bufs=3))
        qp = ctx.enter_context(tc.tile_pool(name="qp", bufs=8))
        hp = ctx.enter_context(tc.tile_pool(name="hp", bufs=8))
        zbp = ctx.enter_context(tc.tile_pool(name="zbp", bufs=6))
        kp = ctx.enter_context(tc.tile_pool(name="kp", bufs=4))
        ph1p = ctx.enter_context(tc.tile_pool(name="ph1p", bufs=2, space="PSUM"))
        ph2p = ctx.enter_context(tc.tile_pool(name="ph2p", bufs=2, space="PSUM"))
        gpp = ctx.enter_context(tc.tile_pool(name="gpp", bufs=2, space="PSUM"))

        dma = nc.sync.dma_start

        def load(pool, ap):
            t = pool.tile(list(ap.shape), ap.dtype, tag=ap.tensor.name)
            dma(out=t[:], in_=ap)
            return t

        wmm = load(singles, wmm_d)
        w1b, w1r = wmm[:, 0:128], wmm[:, 128:256]
        w2b, w2r = wmm[:, 256:384], wmm[:, 384:512]
        w3b, w3r = wmm[:, 512:1536], wmm[:, 1536:2560]
        b3sel = load(singles, b3sel_d)
        b3mat, sel128 = b3sel[:, 0:128], b3sel[:, 128:256]
        b12 = load(singles, b12_d)
        b1mat, b2mat, ones512 = b12[:, 0:128], b12[:, 128:256], b12[:, 256:768]
        wi1x = load(singles, wi1x_d)
        wi1, x0t = wi1x[:, 0:128], wi1x[:, 128:144]
        wi2 = load(singles, wi2_d)
        wi3 = load(singles, wi3_d)
        bi1 = load(singles, bi1_d)
        bi2 = load(singles, bi2_d)
        bi3 = load(singles, bi3_d)
        # raw (non-pool) PSUM scratch for PE warm-keeping dummies; only the
        # PE writes it (in-order), nobody reads it
        dmy = nc.alloc_psum_tensor("dmy", [1, 384], F32).ap()

        mm = nc.tensor.matmul

        # ---- init MLP (fp32): z0 = mlp(x(t0)) ----
        ph_i1 = ph1p.tile([128, BPC], F32, tag="ph1")
        mm(ph_i1[:], wi1, x0t, start=True, stop=True)
        hi1 = singles.tile([128, BPC], F32, tag="hi1")
        nc.scalar.activation(hi1[:], ph_i1[:], AF.Relu, bias=bi1[:])
        ph_i2 = ph2p.tile([128, BPC], F32, tag="ph2")
        mm(ph_i2[:], wi2[:], hi1[:], start=True, stop=True)
        hi2 = singles.tile([128, BPC], F32, tag="hi2")
        nc.scalar.activation(hi2[:], ph_i2[:], AF.Relu, bias=bi2[:])
        ph_i3 = ph1p.tile([64, BPC], F32, tag="ph1")
        mm(ph_i3[:], wi3[:], hi2[:], start=True, stop=True)

        # stage-0 state: ge slots = [z0; 0], zb/zr fp16 linear
        ge_cur = gep.tile([128, 144], F32, tag="ge")
        slot_full = slot_ap(ge_cur)
        nc.vector.tensor_scalar_add(slot_full[0:64], ph_i3[:], bi3[:])
        nc.vector.memset(slot_full[64:128], 0.0)
        zbw_cur = zbp.tile([128, 2 * BPC], F16, tag="zbw")
        nc.vector.tensor_copy(out=zbw_cur[:, 0:BPC], in_=slot_ap(ge_cur))
        nc.vector.scalar_tensor_tensor(
            out=zbw_cur[:, BPC:2 * BPC], in0=zbw_cur[:, 0:BPC], scalar=-1.0,
            in1=slot_ap(ge_cur), op0=OP.mult, op1=OP.add,
        )

        def dup2(ap):
            """output AP repeating the same addresses for moving cols 0:16
            and 16:32 -> one N=32 matmul accumulates both halves."""
            return ap.unsqueeze(1).broadcast_to([128, 2, BPC])

        def relu_split(tag, psum, engine):
            """bias already seeded into psum. One [128,32] tile: cols 0:16
            hb = relu(psum) (fp16), cols 16:32 hr = max(psum,0) - hb."""
            hw = hp.tile([128, 2 * BPC], F16, tag=tag + "hw")
            hb = hw[:, 0:BPC]
            if engine == "scalar":
                nc.scalar.activation(hb, psum[:], AF.Relu, bias=0.0)
            else:
                nc.vector.tensor_scalar_max(out=hb, in0=psum[:], scalar1=0.0)
            nc.vector.scalar_tensor_tensor(
                out=hw[:, BPC:2 * BPC], in0=psum[:], scalar=0.0, in1=hb,
                op0=OP.max, op1=OP.subtract,
            )
            return hw

        stage_scale = [0.5, 0.5, 1.0]

        for t in range(l_steps):
            # wf [128,144] fp32: one replication DMA per half
            wf = wfp.tile([128, 144], F32, tag="wf")
            vbase = vsmall[t]
            for half in range(2):
                src = bass.AP(
                    tensor=vbase.tensor,
                    offset=vbase.offset + 144 * half,
                    ap=[[0, 64], [1, 144]],
                )
                dma(out=wf[64 * half:64 * (half + 1), 0:144], in_=src)

            # output: state at start of step t, fp16 linear copy (host folds)
            dma(out=zall[t], in_=zbw_cur[:, 0:BPC])

            qs = []
            ge_s, zbw_s = ge_cur, zbw_cur
            kacc12 = kacc123 = pfin = None
            for s in range(4):
                # ---- L1 (bias pre-seeded; zb terms first, zr lands late) ----
                ph1 = ph1p.tile([128, BPC], F32, tag="ph1")
                mm(ph1[:].unsqueeze(1).broadcast_to([128, 32, BPC]),
                   b1mat, ones512, start=True, stop=False,
                   skip_group_check=True)
                mm(ph1[:], w1r, zbw_s[:, 0:BPC], start=False, stop=False,
                   skip_group_check=True)
                mm(dup2(ph1[:]), w1b, zbw_s[:], start=False, stop=True,
                   skip_group_check=True)
                # b3 seed slots in while PE would stall on relu1
                gp = gpp.tile([128, 144], F32, tag="gp")
                mm(grid_ap(gp), b3mat, sel128, start=True, stop=False,
                   skip_group_check=True)
                h1w = relu_split("h1", ph1, "vector")
                # RK4 bookkeeping emitted here so its vector-queue position
                # never delays the semaphore signal for the state tiles
                if s == 2:
                    kacc12 = kp.tile([128, BPC], F32, tag="k")
                    nc.vector.scalar_tensor_tensor(
                        out=kacc12[:], in0=qs[1][:], scalar=2.0, in1=qs[0][:],
                        op0=OP.mult, op1=OP.add,
                    )
                elif s == 3:
                    kacc123 = kp.tile([128, BPC], F32, tag="k")
                    nc.vector.scalar_tensor_tensor(
                        out=kacc123[:], in0=qs[2][:], scalar=2.0, in1=kacc12[:],
                        op0=OP.mult, op1=OP.add,
                    )
                    pfin = kp.tile([128, BPC], F32, tag="k")
                    nc.vector.scalar_tensor_tensor(
                        out=pfin[:], in0=kacc123[:], scalar=1.0 / 6.0,
                        in1=slot_ap(ge_cur), op0=OP.mult, op1=OP.add,
                    )

                # ---- L2 ----
                ph2 = ph2p.tile([128, BPC], F32, tag="ph2")
                mm(ph2[:].unsqueeze(1).broadcast_to([128, 32, BPC]),
                   b2mat, ones512, start=True, stop=False,
                   skip_group_check=True)
                mm(ph2[:], w2r, h1w[:, 0:BPC], start=False, stop=False,
                   skip_group_check=True)
                mm(dup2(ph2[:]), w2b, h1w[:], start=False, stop=True,
                   skip_group_check=True)
                h2w = relu_split("h2", ph2, "vector")

                # ---- L3: hb terms first (never stall on h2r) ----
                for c in range(8):
                    mm(chunk_ap(gp, c), w3r[:, c * 128:(c + 1) * 128],
                       h2w[:, 0:BPC], start=False, stop=False,
                       skip_group_check=True)
                for c in range(8):
                    mm(dup2(chunk_ap(gp, c)), w3b[:, c * 128:(c + 1) * 128],
                       h2w[:], start=False, stop=(c == 7),
                       skip_group_check=True)

                # ---- tail (fp32): tanh -> linear mult -> segmented reduce ----
                nc.scalar.activation(grid_ap(ge_s), grid_ap(gp), AF.Tanh,
                                     bias=0.0)
                m = mp.tile([128, 144], F32, tag="m")
                nc.vector.tensor_tensor(
                    out=m[:], in0=ge_s[:, 0:144], in1=wf[:], op=OP.mult,
                )
                q = qp.tile([128, BPC], F32, tag="q")
                nc.vector.tensor_reduce(
                    out=q[:], in_=m[:].rearrange("p (j c) -> p j c", c=9),
                    axis=mybir.AxisListType.X, op=OP.add,
                )
                qs.append(q)

                if s < 3:
                    ge_n = gep.tile([128, 144], F32, tag="ge")
                    zbw_n = zbp.tile([128, 2 * BPC], F16, tag="zbw")
                    # zb first: unblocks next L1 after one op
                    nc.vector.scalar_tensor_tensor(
                        out=zbw_n[:, 0:BPC], in0=q[:], scalar=stage_scale[s],
                        in1=slot_ap(ge_cur), op0=OP.mult, op1=OP.add,
                    )
                    nc.vector.scalar_tensor_tensor(
                        out=slot_ap(ge_n), in0=q[:], scalar=stage_scale[s],
                        in1=slot_ap(ge_cur), op0=OP.mult, op1=OP.add,
                    )
                    nc.vector.scalar_tensor_tensor(
                        out=zbw_n[:, BPC:2 * BPC], in0=zbw_n[:, 0:BPC],
                        scalar=-1.0, in1=slot_ap(ge_n),
                        op0=OP.mult, op1=OP.add,
                    )
                    ge_s, zbw_s = ge_n, zbw_n
                elif s == 3:
                    ge_next = gep.tile([128, 144], F32, tag="ge")
                    zbw_next = zbp.tile([128, 2 * BPC], F16, tag="zbw")
                    nc.vector.scalar_tensor_tensor(
                        out=zbw_next[:, 0:BPC], in0=q[:], scalar=1.0 / 6.0,
                        in1=pfin[:], op0=OP.mult, op1=OP.add,
                    )
                    nc.vector.scalar_tensor_tensor(
                        out=slot_ap(ge_next), in0=q[:], scalar=1.0 / 6.0,
                        in1=pfin[:], op0=OP.mult, op1=OP.add,
                    )
                    nc.vector.scalar_tensor_tensor(
                        out=zbw_next[:, BPC:2 * BPC], in0=zbw_next[:, 0:BPC],
                        scalar=-1.0, in1=slot_ap(ge_next),
                        op0=OP.mult, op1=OP.add,
                    )
            ge_cur, zbw_cur = ge_next, zbw_next

    nc.compile()
    return nc


def _split_f16(w):
    wb = np.asarray(w, np.float32).astype(np.float16)
    wr = (np.asarray(w, np.float32) - wb.astype(np.float32)).astype(np.float16)
    return wb, wr


def _prep_inputs(t, x, dyn_w1, dyn_b1, dyn_w2, dyn_b2, dyn_w3, dyn_b3,
                 init_w1, init_b1, init_w2, init_b2, init_w3, init_b3,
                 l_steps=L):
    x = np.asarray(x, dtype=np.float32)
    x_aug = np.concatenate([x, x[:, -1:]], axis=1)
    v = (x_aug[:, 1:] - x_aug[:, :-1]) / DT  # [B, L, X]
    sv = v.sum(-1)  # [B, L]

    w1s = np.concatenate([dyn_w1, dyn_w1], axis=0).astype(np.float32)
    w3x = np.empty((H, 1024), dtype=np.float32)
    for c in range(8):
        w3x[:, c * 128:(c + 1) * 128] = dyn_w3[:, ORIG_COL[c]]
    b3row = np.asarray(dyn_b3, np.float32)[ORIG_COL]  # [8, 128]

    w1b, w1r = _split_f16(w1s)
    w2b, w2r = _split_f16(np.asarray(dyn_w2, np.float32))
    w3b, w3r = _split_f16(w3x)
    b3b, b3r = _split_f16(b3row)
    # K=128 seed weights: rows 2c/2c+1 hold b3b/b3r for chunk c; the selector
    # moving activates both rows of chunk c at free index f = j*8+c
    b3mat = np.zeros((128, 128), dtype=np.float16)
    sel128 = np.zeros((128, 128), dtype=np.float16)
    for c in range(8):
        b3mat[2 * c] = b3b[c]
        b3mat[2 * c + 1] = b3r[c]
        for j in range(BPC):
            sel128[2 * c, j * 8 + c] = 1.0
            sel128[2 * c + 1, j * 8 + c] = 1.0

    wmm = np.concatenate([w1b, w1r, w2b, w2r, w3b, w3r], axis=1)  # [128, 2560]
    b3sel = np.concatenate([b3mat, sel128], axis=1)               # [128, 256]
    b1b, b1r = _split_f16(np.asarray(dyn_b1, np.float32).reshape(1, 128))
    b2b, b2r = _split_f16(np.asarray(dyn_b2, np.float32).reshape(1, 128))
    b1mat = np.zeros((128, 128), dtype=np.float16)
    b1mat[0], b1mat[1] = b1b / 32, b1r / 32
    b2mat = np.zeros((128, 128), dtype=np.float16)
    b2mat[0], b2mat[1] = b2b / 32, b2r / 32
    b12 = np.concatenate([
        b1mat, b2mat, np.ones((128, 512), dtype=np.float16),
    ], axis=1)                                                    # [128, 768]

    shared = dict(
        wmm=np.ascontiguousarray(wmm), b3sel=np.ascontiguousarray(b3sel),
        b12=np.ascontiguousarray(b12),
        wi2=np.asarray(init_w2, np.float32),
        wi3=np.asarray(init_w3, np.float32),
        bi1=np.asarray(init_b1, np.float32).reshape(128, 1),
        bi2=np.asarray(init_b2, np.float32).reshape(128, 1),
        bi3=np.asarray(init_b3, np.float32).reshape(64, 1),
    )
    wi1 = np.asarray(init_w1, np.float32)

    in_maps = []
    for core in range(NCORES):
        sl = slice(core * BPC, (core + 1) * BPC)
        vb = v[sl, :l_steps]            # [BPC, l, X]
        svb = sv[sl, :l_steps]          # [BPC, l]
        # vsmall row layout: [half h][j*9+c]: c<8 -> dt*v[t, 2c+h, j];
        # c==8 -> -0.001*dt*sum_x v[t, :, j]
        vsm = np.empty((l_steps, 2, BPC, 9), dtype=np.float32)
        dv = DT * vb.transpose(1, 2, 0)               # [l, X, BPC]
        for h in range(2):
            for c in range(8):
                vsm[:, h, :, c] = dv[:, 2 * c + h, :]
        svdc = (-0.001 * DT * svb.T).astype(np.float32)  # [l, BPC]
        vsm[:, 0, :, 8] = svdc
        vsm[:, 1, :, 8] = svdc
        vsm = vsm.reshape(l_steps, 288)
        x0tc = x[sl, 0, :].T.astype(np.float32)          # [X, BPC]
        wi1x = np.concatenate([wi1, x0tc], axis=1)       # [16, 144]
        m = dict(shared)
        m.update(vsmall=np.ascontiguousarray(vsm),
                 wi1x=np.ascontiguousarray(wi1x))
        in_maps.append(m)
    return in_maps


_NC_CACHE = {}


def kernel_traced(trace=False, **inputs):
    key = L
    if key not in _NC_CACHE:
        _NC_CACHE[key] = build_nc(L)
    nc = _NC_CACHE[key]
    in_maps = _prep_inputs(**inputs, l_steps=L)
    res = run_bass_kernel_spmd(nc, in_maps, list(range(NCORES)), trace=trace)
    out = np.empty((B, L, Z), dtype=np.float32)
    for core in range(NCORES):
        zall = res.results[core]["zall"].astype(np.float32)  # [L,128,BPC] split
        zf = zall[:, :Z] + zall[:, Z:]
        out[core * BPC:(core + 1) * BPC] = zf.transpose(2, 0, 1)
    return out, res


def kernel(**inputs):
    return kernel_traced(trace=False, **inputs)[0]


# revision 43
# speedup vs baseline: 1.0007x; 1.0007x over previous
"""Neural CDE (RK4, piecewise-constant path derivative) Trainium2 kernel.

V4: pure batch parallelism, B=128 -> 16 per core across 8 cores.
State kept feature-major in split form (top[64]+bottom[64]) interleaved into
the tail tile: ge[p, j*9+c] holds tanh outputs for chunks c<8 and the fp32
state for c==8, so the q-contraction is one fully-linear multiply + one
linear segmented reduce (9-wide contiguous groups).

Matmuls run in double-fp16, 3 terms: Wb.hb + Wr.hb + Wb.hr (the Wr.hr term
is ~2^-22 relative, dropped). fp16 halves the residual magnitude vs bf16
(2^-11 vs 2^-8), and the 3-term scheme keeps the recurrence at fp32 grade
(CPU-sim rel err 3e-4 over the 512-step horizon vs 2e-2 tolerance).

Per relu boundary the (hb, hr) pair is built with the scalar engine (hb,
fp16 out) in parallel with the vector engine (h32), then hr = h32 - hb; the
next layer's two hb-terms issue as soon as hb lands. The tail is fp32
throughout (any 16-bit rounding there amplifies ~350x over the horizon).
"""

import os
import sys
from contextlib import ExitStack

import numpy as np
import ml_dtypes

sys.path.insert(0, "/opt/trn_rl_repo")

import concourse.bass as bass
import concourse.tile as tile
from concourse import bacc
from concourse import mybir
from concourse.bass_utils import run_bass_kernel_spmd

B, L, X, Z, H = 128, 512, 16, 64, 128
NCORES = 8
BPC = B // NCORES  # 16
DT = 0.1
F32 = mybir.dt.float32
F16 = mybir.dt.float16
AF = mybir.ActivationFunctionType
OP = mybir.AluOpType

# chunk permutation: psum column group c holds original W3 column z*16+x
# with x = 2c + (p>=64), z = p%64 (p = output partition)
_p = np.arange(128)
_c = np.arange(8)
ORIG_COL = (_p[None, :] % 64) * 16 + 2 * _c[:, None] + (_p[None, :] // 64)  # [8,128]


def _v3(t):
    return t[:].rearrange("p (j c) -> p j c", c=9)


def slot_ap(t):
    """strided state-slot view: positions j*9+8 within a [128,144] tile."""
    return _v3(t)[:, :, 8]


def grid_ap(t):
    """3D view of the 128 tanh positions j*9+c (j outer, c inner)."""
    return _v3(t)[:, :, 0:8]


def chunk_ap(t, c):
    """strided chunk view: positions j*9+c, 16 elements."""
    return _v3(t)[:, :, c]


def build_nc(l_steps=L):
    nc = bacc.Bacc("TRN2")

    dp = nc.declare_dram_parameter
    # wf rows: per half h: 144 fp32 values val(h, j*9+c); c<8: dt*v[2c+h, j],
    # c==8: -0.001*dt*sum_x v
    vsmall = dp("vsmall", [l_steps, 288], F32, isOutput=False).ap()
    # [w1b|w1r|w2b|w2r|w3b(1024)|w3r(1024)] all fp16, single DMA
    wmm_d = dp("wmm", [128, 2560], F16, isOutput=False).ap()
    # [B3MAT | SEL128] fp16: K=128 seed weights (rows 2c+t = b3{b,r}[c]) and
    # selector moving (row 2c+t active for chunk c) -> FWL-rate weight loads
    b3sel_d = dp("b3sel", [128, 256], F16, isOutput=False).ap()
    # [B1MAT | B2MAT | ones] fp16: K=128 bias-seed weights (rows 0/1 = bb/br)
    b12_d = dp("b12", [128, 272], F16, isOutput=False).ap()
    wi1x_d = dp("wi1x", [16, 144], F32, isOutput=False).ap()  # [wi1 | x0t]
    wi2_d = dp("wi2", [128, 128], F32, isOutput=False).ap()
    wi3_d = dp("wi3", [128, 64], F32, isOutput=False).ap()
    bi1_d = dp("bi1", [128, 1], F32, isOutput=False).ap()
    bi2_d = dp("bi2", [128, 1], F32, isOutput=False).ap()
    bi3_d = dp("bi3", [64, 1], F32, isOutput=False).ap()
    # split-form state per step (fp16 copy; host folds top+bottom halves)
    zall = dp("zall", [l_steps, 128, BPC], F16, isOutput=True).ap()

    with tile.TileContext(nc) as tc, ExitStack() as ctx:
        singles = ctx.enter_context(tc.tile_pool(name="singles", bufs=1))
        wfp = ctx.enter_context(tc.tile_pool(name="wfp", bufs=4))
        gep = ctx.enter_context(tc.tile_pool(name="gep", bufs=4))
        mp = ctx.enter_context(tc.tile_pool(name="mp", bufs=3))
        qp = ctx.enter_context(tc.tile_pool(name="qp", bufs=8))
        hp = ctx.enter_context(tc.tile_pool(name="hp", bufs=8))
        zbp = ctx.enter_context(tc.tile_pool(name="zbp", bufs=6))
        kp = ctx.enter_context(tc.tile_pool(name="kp", bufs=4))
        ph1p = ctx.enter_context(tc.tile_pool(name="ph1p", bufs=2, space="PSUM"))
        ph2p = ctx.enter_context(tc.tile_pool(name="ph2p", bufs=2, space="PSUM"))
        gpp = ctx.enter_context(tc.tile_pool(name="gpp", bufs=2, space="PSUM"))

        dma = nc.sync.dma_start

        def load(pool, ap):
            t = pool.tile(list(ap.shape), ap.dtype, tag=ap.tensor.name)
            dma(out=t[:], in_=ap)
            return t

        wmm = load(singles, wmm_d)
        w1b, w1r = wmm[:, 0:128], wmm[:, 128:256]
        w2b, w2r = wmm[:, 256:384], wmm[:, 384:512]
        w3b, w3r = wmm[:, 512:1536], wmm[:, 1536:2560]
        b3sel = load(singles, b3sel_d)
        b3mat, sel128 = b3sel[:, 0:128], b3sel[:, 128:256]
        b12 = load(singles, b12_d)
        b1mat, b2mat, ones16 = b12[:, 0:128], b12[:, 128:256], b12[:, 256:272]
        wi1x = load(singles, wi1x_d)
        wi1, x0t = wi1x[:, 0:128], wi1x[:, 128:144]
        wi2 = load(singles, wi2_d)
        wi3 = load(singles, wi3_d)
        bi1 = load(singles, bi1_d)
        bi2 = load(singles, bi2_d)
        bi3 = load(singles, bi3_d)
        # raw (non-pool) PSUM scratch for PE warm-keeping dummies; only the
        # PE writes it (in-order), nobody reads it
        dmy = nc.alloc_psum_tensor("dmy", [1, 384], F32).ap()

        mm = nc.tensor.matmul

        # ---- init MLP (fp32): z0 = mlp(x(t0)) ----
        ph_i1 = ph1p.tile([128, BPC], F32, tag="ph1")
        mm(ph_i1[:], wi1, x0t, start=True, stop=True)
        hi1 = singles.tile([128, BPC], F32, tag="hi1")
        nc.scalar.activation(hi1[:], ph_i1[:], AF.Relu, bias=bi1[:])
        ph_i2 = ph2p.tile([128, BPC], F32, tag="ph2")
        mm(ph_i2[:], wi2[:], hi1[:], start=True, stop=True)
        hi2 = singles.tile([128, BPC], F32, tag="hi2")
        nc.scalar.activation(hi2[:], ph_i2[:], AF.Relu, bias=bi2[:])
        ph_i3 = ph1p.tile([64, BPC], F32, tag="ph1")
        mm(ph_i3[:], wi3[:], hi2[:], start=True, stop=True)

        # stage-0 state: ge slots = [z0; 0], zb/zr fp16 linear
        ge_cur = gep.tile([128, 144], F32, tag="ge")
        slot_full = slot_ap(ge_cur)
        nc.vector.tensor_scalar_add(slot_full[0:64], ph_i3[:], bi3[:])
        nc.vector.memset(slot_full[64:128], 0.0)
        zbw_cur = zbp.tile([128, 2 * BPC], F16, tag="zbw")
        nc.vector.tensor_copy(out=zbw_cur[:, 0:BPC], in_=slot_ap(ge_cur))
        nc.vector.scalar_tensor_tensor(
            out=zbw_cur[:, BPC:2 * BPC], in0=zbw_cur[:, 0:BPC], scalar=-1.0,
            in1=slot_ap(ge_cur), op0=OP.mult, op1=OP.add,
        )

        def dup2(ap):
            """output AP repeating the same addresses for moving cols 0:16
            and 16:32 -> one N=32 matmul accumulates both halves."""
            return ap.unsqueeze(1).broadcast_to([128, 2, BPC])

        def relu_split(tag, psum, engine):
            """bias already seeded into psum. One [128,32] tile: cols 0:16
            hb = relu(psum) (fp16), cols 16:32 hr = max(psum,0) - hb."""
            hw = hp.tile([128, 2 * BPC], F16, tag=tag + "hw")
            hb = hw[:, 0:BPC]
            if engine == "scalar":
                nc.scalar.activation(hb, psum[:], AF.Relu, bias=0.0)
            else:
                nc.vector.tensor_scalar_max(out=hb, in0=psum[:], scalar1=0.0)
            nc.vector.scalar_tensor_tensor(
                out=hw[:, BPC:2 * BPC], in0=psum[:], scalar=0.0, in1=hb,
                op0=OP.max, op1=OP.subtract,
            )
            return hw

        stage_scale = [0.5, 0.5, 1.0]

        for t in range(l_steps):
            # wf [128,144] fp32: one replication DMA per half
            wf = wfp.tile([128, 144], F32, tag="wf")
            vbase = vsmall[t]
            for half in range(2):
                src = bass.AP(
                    tensor=vbase.tensor,
                    offset=vbase.offset + 144 * half,
                    ap=[[0, 64], [1, 144]],
                )
                dma(out=wf[64 * half:64 * (half + 1), 0:144], in_=src)

            # output: state at start of step t, fp16 linear copy (host folds)
            dma(out=zall[t], in_=zbw_cur[:, 0:BPC])

            qs = []
            ge_s, zbw_s = ge_cur, zbw_cur
            kacc12 = kacc123 = pfin = None
            for s in range(4):
                # ---- L1 (bias pre-seeded; zb terms first, zr lands late) ----
                ph1 = ph1p.tile([128, BPC], F32, tag="ph1")
                mm(ph1[:], b1mat, ones16, start=True, stop=False,
                   skip_group_check=True)
                mm(ph1[:], w1r, zbw_s[:, 0:BPC], start=False, stop=False,
                   skip_group_check=True)
                mm(dup2(ph1[:]), w1b, zbw_s[:], start=False, stop=True,
                   skip_group_check=True)
                # b3 seed slots in while PE would stall on relu1
                gp = gpp.tile([128, 144], F32, tag="gp")
                mm(grid_ap(gp), b3mat, sel128, start=True, stop=False,
                   skip_group_check=True)
                h1w = relu_split("h1", ph1, "vector")

                # ---- L2 ----
                ph2 = ph2p.tile([128, BPC], F32, tag="ph2")
                mm(ph2[:], b2mat, ones16, start=True, stop=False,
                   skip_group_check=True)
                mm(ph2[:], w2r, h1w[:, 0:BPC], start=False, stop=False,
                   skip_group_check=True)
                mm(dup2(ph2[:]), w2b, h1w[:], start=False, stop=True,
                   skip_group_check=True)
                h2w = relu_split("h2", ph2, "vector")

                # ---- L3: hb terms first (never stall on h2r) ----
                for c in range(8):
                    mm(chunk_ap(gp, c), w3r[:, c * 128:(c + 1) * 128],
                       h2w[:, 0:BPC], start=False, stop=False,
                       skip_group_check=True)
                for c in range(8):
                    mm(dup2(chunk_ap(gp, c)), w3b[:, c * 128:(c + 1) * 128],
                       h2w[:], start=False, stop=(c == 7),
                       skip_group_check=True)
                # RK4 bookkeeping emitted after both relu splits: its
                # vector-queue slot sits behind hr2, so the semaphore for the
                # state-update stts is never batched behind it
                if s == 2:
                    kacc12 = kp.tile([128, BPC], F32, tag="k")
                    nc.vector.scalar_tensor_tensor(
                        out=kacc12[:], in0=qs[1][:], scalar=2.0, in1=qs[0][:],
                        op0=OP.mult, op1=OP.add,
                    )
                elif s == 3:
                    kacc123 = kp.tile([128, BPC], F32, tag="k")
                    nc.vector.scalar_tensor_tensor(
                        out=kacc123[:], in0=qs[2][:], scalar=2.0, in1=kacc12[:],
                        op0=OP.mult, op1=OP.add,
                    )
                    pfin = kp.tile([128, BPC], F32, tag="k")
                    nc.vector.scalar_tensor_tensor(
                        out=pfin[:], in0=kacc123[:], scalar=1.0 / 6.0,
                        in1=slot_ap(ge_cur), op0=OP.mult, op1=OP.add,
                    )

                # ---- tail (fp32): tanh -> linear mult -> segmented reduce ----
                nc.scalar.activation(grid_ap(ge_s), grid_ap(gp), AF.Tanh,
                                     bias=0.0)
                m = mp.tile([128, 144], F32, tag="m")
                nc.vector.tensor_tensor(
                    out=m[:], in0=ge_s[:, 0:144], in1=wf[:], op=OP.mult,
                )
                q = qp.tile([128, BPC], F32, tag="q")
                nc.vector.tensor_reduce(
                    out=q[:], in_=m[:].rearrange("p (j c) -> p j c", c=9),
                    axis=mybir.AxisListType.X, op=OP.add,
                )
                qs.append(q)

                if s < 3:
                    ge_n = gep.tile([128, 144], F32, tag="ge")
                    zbw_n = zbp.tile([128, 2 * BPC], F16, tag="zbw")
                    # zb first: unblocks next L1 after one op
                    nc.vector.scalar_tensor_tensor(
                        out=zbw_n[:, 0:BPC], in0=q[:], scalar=stage_scale[s],
                        in1=slot_ap(ge_cur), op0=OP.mult, op1=OP.add,
                    )
                    nc.vector.scalar_tensor_tensor(
                        out=slot_ap(ge_n), in0=q[:], scalar=stage_scale[s],
                        in1=slot_ap(ge_cur), op0=OP.mult, op1=OP.add,
                    )
                    nc.vector.scalar_tensor_tensor(
                        out=zbw_n[:, BPC:2 * BPC], in0=zbw_n[:, 0:BPC],
                        scalar=-1.0, in1=slot_ap(ge_n),
                        op0=OP.mult, op1=OP.add,
                    )
                    ge_s, zbw_s = ge_n, zbw_n
                elif s == 3:
                    ge_next = gep.tile([128, 144], F32, tag="ge")
                    zbw_next = zbp.tile([128, 2 * BPC], F16, tag="zbw")
                    nc.vector.scalar_tensor_tensor(
                        out=zbw_next[:, 0:BPC], in0=q[:], scalar=1.0 / 6.0,
                        in1=pfin[:], op0=OP.mult, op1=OP.add,
                    )
                    nc.vector.scalar_tensor_tensor(
                        out=slot_ap(ge_next), in0=q[:], scalar=1.0 / 6.0,
                        in1=pfin[:], op0=OP.mult, op1=OP.add,
                    )
                    nc.vector.scalar_tensor_tensor(
                        out=zbw_next[:, BPC:2 * BPC], in0=zbw_next[:, 0:BPC],
                        scalar=-1.0, in1=slot_ap(ge_next),
                        op0=OP.mult, op1=OP.add,
                    )
            ge_cur, zbw_cur = ge_next, zbw_next

    nc.compile()
    return nc


def _split_f16(w):
    wb = np.asarray(w, np.float32).astype(np.float16)
    wr = (np.asarray(w, np.float32) - wb.astype(np.float32)).astype(np.float16)
    return wb, wr


def _prep_inputs(t, x, dyn_w1, dyn_b1, dyn_w2, dyn_b2, dyn_w3, dyn_b3,
                 init_w1, init_b1, init_w2, init_b2, init_w3, init_b3,
                 l_steps=L):
    x = np.asarray(x, dtype=np.float32)
    x_aug = np.concatenate([x, x[:, -1:]], axis=1)
    v = (x_aug[:, 1:] - x_aug[:, :-1]) / DT  # [B, L, X]
    sv = v.sum(-1)  # [B, L]

    w1s = np.concatenate([dyn_w1, dyn_w1], axis=0).astype(np.float32)
    w3x = np.empty((H, 1024), dtype=np.float32)
    for c in range(8):
        w3x[:, c * 128:(c + 1) * 128] = dyn_w3[:, ORIG_COL[c]]
    b3row = np.asarray(dyn_b3, np.float32)[ORIG_COL]  # [8, 128]

    w1b, w1r = _split_f16(w1s)
    w2b, w2r = _split_f16(np.asarray(dyn_w2, np.float32))
    w3b, w3r = _split_f16(w3x)
    b3b, b3r = _split_f16(b3row)
    # K=128 seed weights: rows 2c/2c+1 hold b3b/b3r for chunk c; the selector
    # moving activates both rows of chunk c at free index f = j*8+c
    b3mat = np.zeros((128, 128), dtype=np.float16)
    sel128 = np.zeros((128, 128), dtype=np.float16)
    for c in range(8):
        b3mat[2 * c] = b3b[c]
        b3mat[2 * c + 1] = b3r[c]
        for j in range(BPC):
            sel128[2 * c, j * 8 + c] = 1.0
            sel128[2 * c + 1, j * 8 + c] = 1.0

    wmm = np.concatenate([w1b, w1r, w2b, w2r, w3b, w3r], axis=1)  # [128, 2560]
    b3sel = np.concatenate([b3mat, sel128], axis=1)               # [128, 256]
    b1b, b1r = _split_f16(np.asarray(dyn_b1, np.float32).reshape(1, 128))
    b2b, b2r = _split_f16(np.asarray(dyn_b2, np.float32).reshape(1, 128))
    b1mat = np.zeros((128, 128), dtype=np.float16)
    b1mat[0], b1mat[1] = b1b, b1r
    b2mat = np.zeros((128, 128), dtype=np.float16)
    b2mat[0], b2mat[1] = b2b, b2r
    b12 = np.concatenate([
        b1mat, b2mat, np.ones((128, 16), dtype=np.float16),
    ], axis=1)                                                    # [128, 272]

    shared = dict(
        wmm=np.ascontiguousarray(wmm), b3sel=np.ascontiguousarray(b3sel),
        b12=np.ascontiguousarray(b12),
        wi2=np.asarray(init_w2, np.float32),
        wi3=np.asarray(init_w3, np.float32),
        bi1=np.asarray(init_b1, np.float32).reshape(128, 1),
        bi2=np.asarray(init_b2, np.float32).reshape(128, 1),
        bi3=np.asarray(init_b3, np.float32).reshape(64, 1),
    )
    wi1 = np.asarray(init_w1, np.float32)

    in_maps = []
    for core in range(NCORES):
        sl = slice(core * BPC, (core + 1) * BPC)
        vb = v[sl, :l_steps]            # [BPC, l, X]
        svb = sv[sl, :l_steps]          # [BPC, l]
        # vsmall row layout: [half h][j*9+c]: c<8 -> dt*v[t, 2c+h, j];
        # c==8 -> -0.001*dt*sum_x v[t, :, j]
        vsm = np.empty((l_steps, 2, BPC, 9), dtype=np.float32)
        dv = DT * vb.transpose(1, 2, 0)               # [l, X, BPC]
        for h in range(2):
            for c in range(8):
                vsm[:, h, :, c] = dv[:, 2 * c + h, :]
        svdc = (-0.001 * DT * svb.T).astype(np.float32)  # [l, BPC]
        vsm[:, 0, :, 8] = svdc
        vsm[:, 1, :, 8] = svdc
        vsm = vsm.reshape(l_steps, 288)
        x0tc = x[sl, 0, :].T.astype(np.float32)          # [X, BPC]
        wi1x = np.concatenate([wi1, x0tc], axis=1)       # [16, 144]
        m = dict(shared)
        m.update(vsmall=np.ascontiguousarray(vsm),
                 wi1x=np.ascontiguousarray(wi1x))
        in_maps.append(m)
    return in_maps


_NC_CACHE = {}


def kernel_traced(trace=False, **inputs):
    key = L
    if key not in _NC_CACHE:
        _NC_CACHE[key] = build_nc(L)
    nc = _NC_CACHE[key]
    in_maps = _prep_inputs(**inputs, l_steps=L)
    res = run_bass_kernel_spmd(nc, in_maps, list(range(NCORES)), trace=trace)
    out = np.empty((B, L, Z), dtype=np.float32)
    for core in range(NCORES):
        zall = res.results[core]["zall"].astype(np.float32)  # [L,128,BPC] split
        zf = zall[:, :Z] + zall[:, Z:]
        out[core * BPC:(core + 1) * BPC] = zf.transpose(2, 0, 1)
    return out, res


def kernel(**inputs):
    return kernel_traced(trace=False, **inputs)[0]
